# revision 21
# baseline (speedup 1.0000x reference)
"""Causal multi-head attention block on 8 Trainium2 NeuronCores.

Problem: x[4,2048,1024] -> qkv proj (16 heads, dh=64) -> causal softmax
attention -> out proj. Sharding: core = (batch, head-half): each core
computes QKV for 8 heads of one batch, attention for those heads, and a
partial O-projection (its 512 input columns of W_o); host sums the two
partials per batch.

Device kernel (identical SPMD program, per-core data):
  - layouts: x.T [d, t] (host pre-transposed), Q.T/K.T computed as
    [o, t] (feature-major), V as [t, o] with a ones-column appended.
  - scores computed transposed: S.T[k_tile, q_span] = K.T_blk^T @ Q.T,
    exp on ScalarE (scale=1/8 folded in; scores are O(1) so no
    max-subtraction needed), diagonal blocks masked with a 0/1
    lower-triangle multiply after exp.
  - P.T @ [V | 1] with P.T stationary uses the full 128x128 PE array and
    accumulates both numerator and softmax denominator in one PSUM tile.
  - O normalized, transposed on PE, then O-proj partial + 0.5*b_o.

All matmuls bf16 with fp32 PSUM accumulation.
"""

import numpy as np
import ml_dtypes

BF16 = ml_dtypes.bfloat16

B, T, D = 4, 2048, 1024
NH, DH = 16, 64
HPC = 8            # heads per core
OC = HPC * DH      # 512: per-core head columns
NT = T // 128      # 16 q/k tiles of 128
ND = D // 128      # 8 d-tiles
N_CORES = 8

_cache = {}


def _build(debug=False):
    import concourse.bass as bass
    import concourse.mybir as mybir
    import concourse.tile as tile
    from concourse import bacc
    from concourse.masks import make_identity

    f32 = mybir.dt.float32
    bf16 = mybir.dt.bfloat16
    Exp = mybir.ActivationFunctionType.Exp

    nc = bacc.Bacc("TRN2", target_bir_lowering=False, debug=False,
                   num_devices=N_CORES)

    xT = nc.declare_dram_parameter("xT", [D, T], bf16, isOutput=False)
    wqk = nc.declare_dram_parameter("wqkT", [D, 2 * OC], bf16, isOutput=False)
    wv = nc.declare_dram_parameter("wvT", [D, OC], bf16, isOutput=False)
    wo = nc.declare_dram_parameter("woT", [OC, D], bf16, isOutput=False)
    bqk = nc.declare_dram_parameter("bqk", [2 * OC, 1], f32, isOutput=False)
    bv = nc.declare_dram_parameter("bv", [1, OC], f32, isOutput=False)
    bo = nc.declare_dram_parameter("bo", [1, D], f32, isOutput=False)
    tri = nc.declare_dram_parameter("tri", [128, 128], bf16, isOutput=False)
    out = nc.declare_dram_parameter("out", [T, D], f32, isOutput=True)
    if debug:
        d_qkt = nc.declare_dram_parameter("d_qkt", [128, ND * T], bf16, isOutput=True)
        d_vp = nc.declare_dram_parameter(
            "d_vp", [128, NT * HPC * 128], bf16, isOutput=True)
        d_ot = nc.declare_dram_parameter(
            "d_ot", [128, (OC // 128) * T], bf16, isOutput=True)

    with tile.TileContext(nc) as tc:
        with (
            tc.tile_pool(name="persist", bufs=1) as persist,
            tc.tile_pool(name="pt", bufs=4) as ptp,
            tc.tile_pool(name="dn", bufs=4) as dnp,
            tc.tile_pool(name="ostage", bufs=3) as ostage,
            tc.tile_pool(name="psS", bufs=2, space="PSUM") as psS,
            tc.tile_pool(name="psO", bufs=4, space="PSUM") as psO,
        ):
            # ---- persistent SBUF tensors ----
            XT = persist.tile([128, ND, T], bf16)          # x.T d-tiles
            WQK = persist.tile([128, ND, 2 * OC], bf16)
            WV = persist.tile([128, ND, OC], bf16)
            WO = persist.tile([128, OC // 128, D], bf16)
            BQK = persist.tile([128, ND, 1], f32)
            BV = persist.tile([128, OC], f32)
            BO = persist.tile([128, D], f32)
            TRI = persist.tile([128, 128], bf16)
            QKT = persist.tile([128, ND, T], bf16)         # [o, t] Q.T|K.T
            # V' per head, 128 cols: even h: [V(64) | 1*64]; odd h:
            # [1*64 | V(64)]. O.T rows land on partitions (h%2)*64..+64 and
            # the other 64 rows all become the softmax denominator (the
            # matmul broadcasts it for free).
            VP = persist.tile([128, NT, HPC, 128], bf16)
            OT = persist.tile([128, OC // 128, T], bf16)   # attn out.T [c, t]

            nc.sync.dma_start(out=XT[:], in_=xT.rearrange("(n p) t -> p n t", p=128))
            nc.sync.dma_start(out=WQK[:], in_=wqk.rearrange("(n p) o -> p n o", p=128))
            nc.sync.dma_start(out=WV[:], in_=wv.rearrange("(n p) o -> p n o", p=128))
            nc.sync.dma_start(out=WO[:], in_=wo.rearrange("(n p) o -> p n o", p=128))
            nc.sync.dma_start(out=BQK[:], in_=bqk.rearrange("(n p) o -> p n o", p=128))
            nc.gpsimd.dma_start(out=BV[:], in_=bv[:, :].to_broadcast((128, OC)))
            nc.gpsimd.dma_start(out=BO[:], in_=bo[:, :].to_broadcast((128, D)))
            nc.sync.dma_start(out=TRI[:], in_=tri[:, :])
            nc.vector.memset(VP[:, :, 0:HPC:2, DH:128], 1.0)
            nc.vector.memset(VP[:, :, 1:HPC:2, 0:DH], 1.0)

            # ---- QK.T: [o, t] = W_qk @ x.T  (o-tiles: 4 Q then 4 K) ----
            for ot in range(2 * OC // 128):
                for tch in range(T // 512):
                    ps = psS.tile([128, 1024], f32, tag="s")
                    for kd in range(ND):
                        nc.tensor.matmul(
                            ps[:, 0:512],
                            lhsT=WQK[:, kd, ot * 128:(ot + 1) * 128],
                            rhs=XT[:, kd, tch * 512:(tch + 1) * 512],
                            start=(kd == 0), stop=(kd == ND - 1),
                        )
                    nc.vector.tensor_scalar_add(
                        QKT[:, ot, tch * 512:(tch + 1) * 512], ps[:, 0:512],
                        BQK[:, ot, 0:1],
                    )

            # ---- V: [t, o] = x @ W_v.T, bias, ones col stays 1 ----
            for tt in range(NT):
                ps = psS.tile([128, 1024], f32, tag="s")
                for kd in range(ND):
                    nc.tensor.matmul(
                        ps[:, 0:OC],
                        lhsT=XT[:, kd, tt * 128:(tt + 1) * 128],
                        rhs=WV[:, kd, :],
                        start=(kd == 0), stop=(kd == ND - 1),
                    )
                nc.vector.tensor_tensor(
                    out=VP[:, tt, 0:HPC:2, 0:DH],
                    in0=ps[:, 0:OC].rearrange("p (a b) -> p a b", b=DH)[:, 0:HPC:2, :],
                    in1=BV[:].rearrange("p (a b) -> p a b", b=DH)[:, 0:HPC:2, :],
                    op=mybir.AluOpType.add,
                )
                nc.vector.tensor_tensor(
                    out=VP[:, tt, 1:HPC:2, DH:2 * DH],
                    in0=ps[:, 0:OC].rearrange("p (a b) -> p a b", b=DH)[:, 1:HPC:2, :],
                    in1=BV[:].rearrange("p (a b) -> p a b", b=DH)[:, 1:HPC:2, :],
                    op=mybir.AluOpType.add,
                )

            # ---- attention per head; O.T accumulated with V' stationary ----
            for h in range(HPC):
                prow = (h % 2) * 64     # partition row of this head's O.T
                drow = 64 - prow        # denominator rows in OTr
                QTh = QKT[prow:prow + 64, h // 2, :]       # [64, T]
                KTh = QKT[prow:prow + 64, 4 + h // 2, :]   # [64, T]
                for J in range(T // 512):                  # q superblock of 512
                    otr = psO.tile([128, 512], f32, tag="o",
                                   name=f"otr{h}_{J}")
                    ks = list(range(4 * J + 4))
                    pairs = [ks[m:m + 2] for m in range(0, len(ks), 2)]
                    for pair in pairs:
                        # pack S.T of both k-tiles side by side, one exp
                        ps = psS.tile([128, 1024], f32, tag="s")
                        pt = ptp.tile([128, 1024], bf16, tag="p")
                        col = 0
                        offs = []
                        for i in pair:
                            qlo = max(J * 512, i * 128)
                            span = (J + 1) * 512 - qlo
                            # each matmul region must stay within one bank
                            assert col // 512 == (col + span - 1) // 512
                            nc.tensor.matmul(
                                ps[:, col:col + span],
                                lhsT=KTh[:, i * 128:(i + 1) * 128],
                                rhs=QTh[:, qlo:qlo + span],
                                start=True, stop=True,
                            )
                            offs.append((i, col, qlo, span))
                            col += span
                        nc.scalar.activation(
                            out=pt[:, 0:col], in_=ps[:, 0:col],
                            func=Exp, scale=0.125,
                        )
                        for i, coff, qlo, span in offs:
                            if i >= 4 * J:  # diagonal: zero upper triangle
                                nc.vector.tensor_tensor(
                                    out=pt[:, coff:coff + 128],
                                    in0=pt[:, coff:coff + 128], in1=TRI[:],
                                    op=mybir.AluOpType.mult,
                                )
                            # O.T[:, qloc:512] += V'_i.T @ P.T_i
                            qloc = qlo - J * 512
                            nc.tensor.matmul(
                                otr[:, qloc:512],
                                lhsT=VP[:, i, h, :],
                                rhs=pt[:, coff:coff + span],
                                start=(i == 0), stop=(i == 4 * J + 3),
                            )
                    # normalize: O.T rows (base prow) times 1/den rows
                    # (base drow; all 64 identical by construction)
                    rd = dnp.tile([128, 512], f32, tag="d")
                    nc.vector.reciprocal(
                        rd[drow:drow + 64, :], otr[drow:drow + 64, :])
                    nc.vector.tensor_tensor(
                        out=OT[prow:prow + 64, h // 2, J * 512:(J + 1) * 512],
                        in0=otr[prow:prow + 64, :],
                        in1=rd[drow:drow + 64, :],
                        op=mybir.AluOpType.mult,
                    )

            # ---- O-proj partial: out = O @ WoT + 0.5 b_o ----
            for tq in range(NT):
                for oc2 in range(D // 512):
                    ps = psS.tile([128, 1024], f32, tag="s")
                    for ct in range(OC // 128):
                        nc.tensor.matmul(
                            ps[:, 0:512],
                            lhsT=OT[:, ct, tq * 128:(tq + 1) * 128],
                            rhs=WO[:, ct, oc2 * 512:(oc2 + 1) * 512],
                            start=(ct == 0), stop=(ct == OC // 128 - 1),
                        )
                    ob = ostage.tile([128, 512], f32, tag="ob")
                    nc.vector.tensor_tensor(
                        out=ob[:], in0=ps[:, 0:512],
                        in1=BO[:, oc2 * 512:(oc2 + 1) * 512],
                        op=mybir.AluOpType.add,
                    )
                    nc.sync.dma_start(
                        out=out[tq * 128:(tq + 1) * 128, oc2 * 512:(oc2 + 1) * 512],
                        in_=ob[:],
                    )

            if debug:
                nc.sync.dma_start(
                    out=d_qkt[:, :], in_=QKT[:].rearrange("p a t -> p (a t)"))
                nc.sync.dma_start(
                    out=d_vp[:, :], in_=VP[:].rearrange("p a b c -> p (a b c)"))
                nc.sync.dma_start(
                    out=d_ot[:, :], in_=OT[:].rearrange("p a t -> p (a t)"))

    nc.compile()
    return nc


def _in_maps(x, W_qkv, b_qkv, W_o, b_o):
    x = np.asarray(x, np.float32)
    W_qkv = np.asarray(W_qkv, np.float32)
    b_qkv = np.asarray(b_qkv, np.float32)
    W_o = np.asarray(W_o, np.float32)
    b_o = np.asarray(b_o, np.float32)

    maps = []
    for c in range(N_CORES):
        b, hh = c // 2, c % 2
        rs = slice(hh * OC, (hh + 1) * OC)
        wq = W_qkv[0 * D:1 * D][rs]            # [512, 1024]
        wk = W_qkv[1 * D:2 * D][rs]
        wv = W_qkv[2 * D:3 * D][rs]
        wqkT = np.concatenate([wq, wk], 0).T   # [1024, 1024]
        bq = b_qkv[0 * D:1 * D][rs]
        bk = b_qkv[1 * D:2 * D][rs]
        bvv = b_qkv[2 * D:3 * D][rs]
        tri = np.triu(np.ones((128, 128), np.float32))
        maps.append({
            "xT": np.ascontiguousarray(x[b].T).astype(BF16),
            "wqkT": np.ascontiguousarray(wqkT).astype(BF16),
            "wvT": np.ascontiguousarray(wv.T).astype(BF16),
            "woT": np.ascontiguousarray(W_o[:, rs].T).astype(BF16),
            "bqk": np.concatenate([bq, bk]).reshape(2 * OC, 1),
            "bv": bvv.reshape(1, OC),
            "bo": (0.5 * b_o).reshape(1, D),
            "tri": tri.astype(BF16),
        })
    return maps


def _run(x, W_qkv, b_qkv, W_o, b_o, trace=False, tmpdir=None):
    from concourse.bass_utils import run_bass_kernel_spmd

    if "nc" not in _cache:
        _cache["nc"] = _build()
    res = run_bass_kernel_spmd(
        _cache["nc"], _in_maps(x, W_qkv, b_qkv, W_o, b_o),
        core_ids=list(range(N_CORES)), trace=trace, tmpdir=tmpdir,
    )
    out = np.empty((B, T, D), np.float32)
    for b in range(B):
        out[b] = res.results[2 * b]["out"] + res.results[2 * b + 1]["out"]
    return out, res


def kernel(x, W_qkv, b_qkv, W_o, b_o):
    out, _ = _run(x, W_qkv, b_qkv, W_o, b_o, trace=False)
    return out


# revision 26
# speedup vs baseline: 1.3228x; 1.3228x over previous
"""Causal multi-head attention block on 8 Trainium2 NeuronCores.

Problem: x[4,2048,1024] -> qkv proj (16 heads, dh=64) -> causal softmax
attention -> out proj. Sharding: core = (batch, head-half): each core
computes QKV for 8 heads of one batch, attention for those heads, and a
partial O-projection (its 512 input columns of W_o); host sums the two
partials per batch.

Device kernel (identical SPMD program, per-core data):
  - layouts: x.T [d, t] (host pre-transposed), Q.T/K.T computed as
    [o, t] (feature-major), V as [t, o] with a ones-column appended.
  - scores computed transposed: S.T[k_tile, q_span] = K.T_blk^T @ Q.T,
    exp on ScalarE (scale=1/8 folded in; scores are O(1) so no
    max-subtraction needed), diagonal blocks masked with a 0/1
    lower-triangle multiply after exp.
  - P.T @ [V | 1] with P.T stationary uses the full 128x128 PE array and
    accumulates both numerator and softmax denominator in one PSUM tile.
  - O normalized, transposed on PE, then O-proj partial + 0.5*b_o.

All matmuls bf16 with fp32 PSUM accumulation.
"""

import numpy as np
import ml_dtypes

BF16 = ml_dtypes.bfloat16

B, T, D = 4, 2048, 1024
NH, DH = 16, 64
HPC = 8            # heads per core
OC = HPC * DH      # 512: per-core head columns
NT = T // 128      # 16 q/k tiles of 128
ND = D // 128      # 8 d-tiles
N_CORES = 8

_cache = {}


def _build(debug=False):
    import concourse.bass as bass
    import concourse.mybir as mybir
    import concourse.tile as tile
    from concourse import bacc
    from concourse.masks import make_identity

    f32 = mybir.dt.float32
    bf16 = mybir.dt.bfloat16
    Exp = mybir.ActivationFunctionType.Exp

    nc = bacc.Bacc("TRN2", target_bir_lowering=False, debug=False,
                   num_devices=N_CORES)

    xT = nc.declare_dram_parameter("xT", [D, T], bf16, isOutput=False)
    wqk = nc.declare_dram_parameter("wqkT", [D, 2 * OC], bf16, isOutput=False)
    wv = nc.declare_dram_parameter("wvT", [D, OC], bf16, isOutput=False)
    wo = nc.declare_dram_parameter("woT", [OC, D], bf16, isOutput=False)
    bqk = nc.declare_dram_parameter("bqk", [2 * OC, 1], f32, isOutput=False)
    bv = nc.declare_dram_parameter("bv", [1, OC], f32, isOutput=False)
    bo = nc.declare_dram_parameter("bo", [1, D], f32, isOutput=False)
    tri = nc.declare_dram_parameter("tri", [128, 128], bf16, isOutput=False)
    out = nc.declare_dram_parameter("out", [T, D], f32, isOutput=True)
    if debug:
        d_qkt = nc.declare_dram_parameter("d_qkt", [128, ND * T], bf16, isOutput=True)
        d_vp = nc.declare_dram_parameter(
            "d_vp", [128, NT * HPC * 128], bf16, isOutput=True)
        d_ot = nc.declare_dram_parameter(
            "d_ot", [128, (OC // 128) * T], bf16, isOutput=True)

    with tile.TileContext(nc) as tc:
        with (
            tc.tile_pool(name="persist", bufs=1) as persist,
            tc.tile_pool(name="pt", bufs=4) as ptp,
            tc.tile_pool(name="dn", bufs=4) as dnp,
            tc.tile_pool(name="ostage", bufs=3) as ostage,
            tc.tile_pool(name="psS", bufs=3, space="PSUM") as psS,
            tc.tile_pool(name="psO", bufs=2, space="PSUM") as psO,
        ):
            # ---- persistent SBUF tensors ----
            XT = persist.tile([128, ND, T], bf16)          # x.T d-tiles
            WQK = persist.tile([128, ND, 2 * OC], bf16)
            WV = persist.tile([128, ND, OC], bf16)
            WO = persist.tile([128, OC // 128, D], bf16)
            BQK = persist.tile([128, ND, 1], f32)
            BV = persist.tile([128, OC], f32)
            BO = persist.tile([128, D], f32)
            TRI = persist.tile([128, 128], bf16)
            QKT = persist.tile([128, ND, T], bf16)         # [o, t] Q.T|K.T
            # V' per head, 128 cols: even h: [V(64) | 1*64]; odd h:
            # [1*64 | V(64)]. O.T rows land on partitions (h%2)*64..+64 and
            # the other 64 rows all become the softmax denominator (the
            # matmul broadcasts it for free).
            VP = persist.tile([128, NT, HPC, 128], bf16)
            OT = persist.tile([128, OC // 128, T], bf16)   # attn out.T [c, t]

            nc.sync.dma_start(out=XT[:], in_=xT.rearrange("(n p) t -> p n t", p=128))
            nc.sync.dma_start(out=WQK[:], in_=wqk.rearrange("(n p) o -> p n o", p=128))
            nc.sync.dma_start(out=WV[:], in_=wv.rearrange("(n p) o -> p n o", p=128))
            nc.sync.dma_start(out=WO[:], in_=wo.rearrange("(n p) o -> p n o", p=128))
            nc.sync.dma_start(out=BQK[:], in_=bqk.rearrange("(n p) o -> p n o", p=128))
            nc.gpsimd.dma_start(out=BV[:], in_=bv[:, :].to_broadcast((128, OC)))
            nc.gpsimd.dma_start(out=BO[:], in_=bo[:, :].to_broadcast((128, D)))
            nc.sync.dma_start(out=TRI[:], in_=tri[:, :])
            nc.vector.memset(VP[:, :, 0:HPC:2, DH:128], 1.0)
            nc.vector.memset(VP[:, :, 1:HPC:2, 0:DH], 1.0)

            # ---- QK.T: [o, t] = W_qk @ x.T  (o-tiles: 4 Q then 4 K) ----
            for ot in range(2 * OC // 128):
                for tch in range(T // 512):
                    ps = psS.tile([128, 1024], f32, tag="s")
                    for kd in range(ND):
                        nc.tensor.matmul(
                            ps[:, 0:512],
                            lhsT=WQK[:, kd, ot * 128:(ot + 1) * 128],
                            rhs=XT[:, kd, tch * 512:(tch + 1) * 512],
                            start=(kd == 0), stop=(kd == ND - 1),
                        )
                    nc.vector.tensor_scalar_add(
                        QKT[:, ot, tch * 512:(tch + 1) * 512], ps[:, 0:512],
                        BQK[:, ot, 0:1],
                    )

            # ---- V: [t, o] = x @ W_v.T, bias, ones col stays 1 ----
            for tt in range(NT):
                ps = psS.tile([128, 1024], f32, tag="s")
                for kd in range(ND):
                    nc.tensor.matmul(
                        ps[:, 0:OC],
                        lhsT=XT[:, kd, tt * 128:(tt + 1) * 128],
                        rhs=WV[:, kd, :],
                        start=(kd == 0), stop=(kd == ND - 1),
                    )
                nc.vector.tensor_tensor(
                    out=VP[:, tt, 0:HPC:2, 0:DH],
                    in0=ps[:, 0:OC].rearrange("p (a b) -> p a b", b=DH)[:, 0:HPC:2, :],
                    in1=BV[:].rearrange("p (a b) -> p a b", b=DH)[:, 0:HPC:2, :],
                    op=mybir.AluOpType.add,
                )
                nc.vector.tensor_tensor(
                    out=VP[:, tt, 1:HPC:2, DH:2 * DH],
                    in0=ps[:, 0:OC].rearrange("p (a b) -> p a b", b=DH)[:, 1:HPC:2, :],
                    in1=BV[:].rearrange("p (a b) -> p a b", b=DH)[:, 1:HPC:2, :],
                    op=mybir.AluOpType.add,
                )

            # ---- attention per head; O.T accumulated with V' stationary ----
            # two heads (one even, one odd) are software-pipelined: while
            # ScalarE exps head A's scores, PE runs head B's score matmuls.
            def st_exp(h, J, pair):
                prow = (h % 2) * 64
                QTh = QKT[prow:prow + 64, h // 2, :]
                KTh = QKT[prow:prow + 64, 4 + h // 2, :]
                ps = psS.tile([128, 1024], f32, tag="s",
                              name=f"ps{h}_{J}_{pair[0]}")
                pt = ptp.tile([128, 1024], bf16, tag="p",
                              name=f"pt{h}_{J}_{pair[0]}")
                col = 0
                offs = []
                for i in pair:
                    qlo = max(J * 512, i * 128)
                    span = (J + 1) * 512 - qlo
                    # each matmul region must stay within one bank
                    assert col // 512 == (col + span - 1) // 512
                    nc.tensor.matmul(
                        ps[:, col:col + span],
                        lhsT=KTh[:, i * 128:(i + 1) * 128],
                        rhs=QTh[:, qlo:qlo + span],
                        start=True, stop=True,
                    )
                    offs.append((i, col, qlo, span))
                    col += span
                nc.scalar.activation(
                    out=pt[:, 0:col], in_=ps[:, 0:col], func=Exp, scale=0.125)
                return pt, offs

            def av(h, J, pt, offs, otr):
                for i, coff, qlo, span in offs:
                    if i >= 4 * J:  # diagonal: zero upper triangle
                        nc.vector.tensor_tensor(
                            out=pt[:, coff:coff + 128],
                            in0=pt[:, coff:coff + 128], in1=TRI[:],
                            op=mybir.AluOpType.mult,
                        )
                    # O.T[:, qloc:512] += V'_i.T @ P.T_i
                    qloc = qlo - J * 512
                    nc.tensor.matmul(
                        otr[:, qloc:512],
                        lhsT=VP[:, i, h, :],
                        rhs=pt[:, coff:coff + span],
                        start=(i == 0), stop=(i == 4 * J + 3),
                    )

            def normalize(h, J, otr):
                # O.T rows (base prow) times 1/den rows (base drow; all 64
                # denominator rows are identical by construction)
                prow = (h % 2) * 64
                drow = 64 - prow
                rdc = dnp.tile([128, 512], f32, tag="dc", name=f"rdc{h}_{J}")
                rd = dnp.tile([128, 512], f32, tag="d", name=f"rd{h}_{J}")
                # reciprocal_approx_fast only works at partition base 0
                nc.vector.tensor_copy(
                    rdc[0:64, :], otr[drow:drow + 64, :])
                nc.vector.reciprocal_approx_fast(
                    rd[0:64, :], rdc[0:64, :])
                nc.vector.tensor_tensor(
                    out=OT[prow:prow + 64, h // 2, J * 512:(J + 1) * 512],
                    in0=otr[prow:prow + 64, :],
                    in1=rd[0:64, :],
                    op=mybir.AluOpType.mult,
                )

            for hp in range(HPC // 2):
                h0, h1 = 2 * hp, 2 * hp + 1
                for J in range(T // 512):
                    otr0 = psO.tile([128, 512], f32, tag="o",
                                    name=f"otr{h0}_{J}")
                    otr1 = psO.tile([128, 512], f32, tag="o",
                                    name=f"otr{h1}_{J}")
                    ks = list(range(4 * J + 4))
                    pairs = [ks[m:m + 2] for m in range(0, len(ks), 2)]
                    for pair in pairs:
                        pt0, offs0 = st_exp(h0, J, pair)
                        pt1, offs1 = st_exp(h1, J, pair)
                        av(h0, J, pt0, offs0, otr0)
                        av(h1, J, pt1, offs1, otr1)
                    normalize(h0, J, otr0)
                    normalize(h1, J, otr1)

            # ---- O-proj partial: out = O @ WoT + 0.5 b_o ----
            for tq in range(NT):
                for oc2 in range(D // 512):
                    ps = psS.tile([128, 1024], f32, tag="s")
                    for ct in range(OC // 128):
                        nc.tensor.matmul(
                            ps[:, 0:512],
                            lhsT=OT[:, ct, tq * 128:(tq + 1) * 128],
                            rhs=WO[:, ct, oc2 * 512:(oc2 + 1) * 512],
                            start=(ct == 0), stop=(ct == OC // 128 - 1),
                        )
                    ob = ostage.tile([128, 512], f32, tag="ob")
                    nc.vector.tensor_tensor(
                        out=ob[:], in0=ps[:, 0:512],
                        in1=BO[:, oc2 * 512:(oc2 + 1) * 512],
                        op=mybir.AluOpType.add,
                    )
                    nc.sync.dma_start(
                        out=out[tq * 128:(tq + 1) * 128, oc2 * 512:(oc2 + 1) * 512],
                        in_=ob[:],
                    )

            if debug:
                nc.sync.dma_start(
                    out=d_qkt[:, :], in_=QKT[:].rearrange("p a t -> p (a t)"))
                nc.sync.dma_start(
                    out=d_vp[:, :], in_=VP[:].rearrange("p a b c -> p (a b c)"))
                nc.sync.dma_start(
                    out=d_ot[:, :], in_=OT[:].rearrange("p a t -> p (a t)"))

    nc.compile()
    return nc


def _in_maps(x, W_qkv, b_qkv, W_o, b_o):
    x = np.asarray(x, np.float32)
    W_qkv = np.asarray(W_qkv, np.float32)
    b_qkv = np.asarray(b_qkv, np.float32)
    W_o = np.asarray(W_o, np.float32)
    b_o = np.asarray(b_o, np.float32)

    maps = []
    for c in range(N_CORES):
        b, hh = c // 2, c % 2
        rs = slice(hh * OC, (hh + 1) * OC)
        wq = W_qkv[0 * D:1 * D][rs]            # [512, 1024]
        wk = W_qkv[1 * D:2 * D][rs]
        wv = W_qkv[2 * D:3 * D][rs]
        wqkT = np.concatenate([wq, wk], 0).T   # [1024, 1024]
        bq = b_qkv[0 * D:1 * D][rs]
        bk = b_qkv[1 * D:2 * D][rs]
        bvv = b_qkv[2 * D:3 * D][rs]
        tri = np.triu(np.ones((128, 128), np.float32))
        maps.append({
            "xT": np.ascontiguousarray(x[b].T).astype(BF16),
            "wqkT": np.ascontiguousarray(wqkT).astype(BF16),
            "wvT": np.ascontiguousarray(wv.T).astype(BF16),
            "woT": np.ascontiguousarray(W_o[:, rs].T).astype(BF16),
            "bqk": np.concatenate([bq, bk]).reshape(2 * OC, 1),
            "bv": bvv.reshape(1, OC),
            "bo": (0.5 * b_o).reshape(1, D),
            "tri": tri.astype(BF16),
        })
    return maps


def _run(x, W_qkv, b_qkv, W_o, b_o, trace=False, tmpdir=None):
    from concourse.bass_utils import run_bass_kernel_spmd

    if "nc" not in _cache:
        _cache["nc"] = _build()
    res = run_bass_kernel_spmd(
        _cache["nc"], _in_maps(x, W_qkv, b_qkv, W_o, b_o),
        core_ids=list(range(N_CORES)), trace=trace, tmpdir=tmpdir,
    )
    out = np.empty((B, T, D), np.float32)
    for b in range(B):
        out[b] = res.results[2 * b]["out"] + res.results[2 * b + 1]["out"]
    return out, res


def kernel(x, W_qkv, b_qkv, W_o, b_o):
    out, _ = _run(x, W_qkv, b_qkv, W_o, b_o, trace=False)
    return out


# revision 28
# speedup vs baseline: 1.3720x; 1.0373x over previous
"""Causal multi-head attention block on 8 Trainium2 NeuronCores.

Problem: x[4,2048,1024] -> qkv proj (16 heads, dh=64) -> causal softmax
attention -> out proj. Sharding: core = (batch, head-half): each core
computes QKV for 8 heads of one batch, attention for those heads, and a
partial O-projection (its 512 input columns of W_o); host sums the two
partials per batch.

Device kernel (identical SPMD program, per-core data):
  - layouts: x.T [d, t] (host pre-transposed), Q.T/K.T computed as
    [o, t] (feature-major), V as [t, o] with a ones-column appended.
  - scores computed transposed: S.T[k_tile, q_span] = K.T_blk^T @ Q.T,
    exp on ScalarE (scale=1/8 folded in; scores are O(1) so no
    max-subtraction needed), diagonal blocks masked with a 0/1
    lower-triangle multiply after exp.
  - P.T @ [V | 1] with P.T stationary uses the full 128x128 PE array and
    accumulates both numerator and softmax denominator in one PSUM tile.
  - O normalized, transposed on PE, then O-proj partial + 0.5*b_o.

All matmuls bf16 with fp32 PSUM accumulation.
"""

import numpy as np
import ml_dtypes

BF16 = ml_dtypes.bfloat16

B, T, D = 4, 2048, 1024
NH, DH = 16, 64
HPC = 8            # heads per core
OC = HPC * DH      # 512: per-core head columns
NT = T // 128      # 16 q/k tiles of 128
ND = D // 128      # 8 d-tiles
N_CORES = 8

_cache = {}


def _build(debug=False):
    import concourse.bass as bass
    import concourse.mybir as mybir
    import concourse.tile as tile
    from concourse import bacc
    from concourse.masks import make_identity

    f32 = mybir.dt.float32
    bf16 = mybir.dt.bfloat16
    Exp = mybir.ActivationFunctionType.Exp

    nc = bacc.Bacc("TRN2", target_bir_lowering=False, debug=False,
                   num_devices=N_CORES)

    xT = nc.declare_dram_parameter("xT", [D, T], bf16, isOutput=False)
    wqk = nc.declare_dram_parameter("wqkT", [D, 2 * OC], bf16, isOutput=False)
    wv = nc.declare_dram_parameter("wvT", [D, OC], bf16, isOutput=False)
    wo = nc.declare_dram_parameter("woT", [OC, D], bf16, isOutput=False)
    bqk = nc.declare_dram_parameter("bqk", [2 * OC, 1], f32, isOutput=False)
    bv = nc.declare_dram_parameter("bv", [1, OC], f32, isOutput=False)
    bo = nc.declare_dram_parameter("bo", [1, D], f32, isOutput=False)
    tri = nc.declare_dram_parameter("tri", [128, 128], bf16, isOutput=False)
    out = nc.declare_dram_parameter("out", [T, D], f32, isOutput=True)
    if debug:
        d_qkt = nc.declare_dram_parameter("d_qkt", [128, ND * T], bf16, isOutput=True)
        d_vp = nc.declare_dram_parameter(
            "d_vp", [128, NT * HPC * 128], bf16, isOutput=True)
        d_ot = nc.declare_dram_parameter(
            "d_ot", [128, (OC // 128) * T], bf16, isOutput=True)

    with tile.TileContext(nc) as tc:
        with (
            tc.tile_pool(name="persist", bufs=1) as persist,
            tc.tile_pool(name="pt", bufs=4) as ptp,
            tc.tile_pool(name="dn", bufs=4) as dnp,
            tc.tile_pool(name="ostage", bufs=3) as ostage,
            tc.tile_pool(name="psS", bufs=2, space="PSUM") as psS,
            tc.tile_pool(name="psQ", bufs=2, space="PSUM") as psQ,
            tc.tile_pool(name="psO", bufs=2, space="PSUM") as psO,
        ):
            # ---- persistent SBUF tensors ----
            XT = persist.tile([128, ND, T], bf16)          # x.T d-tiles
            WQK = persist.tile([128, ND, 2 * OC], bf16)
            WV = persist.tile([128, ND, OC], bf16)
            WO = persist.tile([128, OC // 128, D], bf16)
            BQK = persist.tile([128, ND, 1], f32)
            BV = persist.tile([128, OC], f32)
            BO = persist.tile([128, D], f32)
            TRI = persist.tile([128, 128], bf16)
            QKT = persist.tile([128, ND, T], bf16)         # [o, t] Q.T|K.T
            # V' per head, 128 cols: even h: [V(64) | 1*64]; odd h:
            # [1*64 | V(64)]. O.T rows land on partitions (h%2)*64..+64 and
            # the other 64 rows all become the softmax denominator (the
            # matmul broadcasts it for free).
            VP = persist.tile([128, NT, HPC, 128], bf16)
            OT = persist.tile([128, OC // 128, T], bf16)   # attn out.T [c, t]

            nc.sync.dma_start(out=XT[:], in_=xT.rearrange("(n p) t -> p n t", p=128))
            nc.sync.dma_start(out=WQK[:], in_=wqk.rearrange("(n p) o -> p n o", p=128))
            nc.sync.dma_start(out=WV[:], in_=wv.rearrange("(n p) o -> p n o", p=128))
            nc.sync.dma_start(out=WO[:], in_=wo.rearrange("(n p) o -> p n o", p=128))
            nc.sync.dma_start(out=BQK[:], in_=bqk.rearrange("(n p) o -> p n o", p=128))
            nc.gpsimd.dma_start(out=BV[:], in_=bv[:, :].to_broadcast((128, OC)))
            nc.gpsimd.dma_start(out=BO[:], in_=bo[:, :].to_broadcast((128, D)))
            nc.sync.dma_start(out=TRI[:], in_=tri[:, :])
            nc.vector.memset(VP[:, :, 0:HPC:2, DH:128], 1.0)
            nc.vector.memset(VP[:, :, 1:HPC:2, 0:DH], 1.0)

            # ---- QKV projection, emitted as fill-in units ----
            def emit_qk(ot, tch):
                # one [o, t] chunk: [128 o, 512 t] = W_qk @ x.T + b
                ps = psQ.tile([128, 512], f32, tag="q",
                              name=f"qk{ot}_{tch}")
                for kd in range(ND):
                    nc.tensor.matmul(
                        ps[:],
                        lhsT=WQK[:, kd, ot * 128:(ot + 1) * 128],
                        rhs=XT[:, kd, tch * 512:(tch + 1) * 512],
                        start=(kd == 0), stop=(kd == ND - 1),
                    )
                nc.vector.tensor_scalar_add(
                    QKT[:, ot, tch * 512:(tch + 1) * 512], ps[:],
                    BQK[:, ot, 0:1],
                )

            def emit_v(tt):
                # one [t, o] tile of V = x @ W_v.T + b, into parity layout
                ps = psQ.tile([128, 512], f32, tag="q", name=f"v{tt}")
                for kd in range(ND):
                    nc.tensor.matmul(
                        ps[:],
                        lhsT=XT[:, kd, tt * 128:(tt + 1) * 128],
                        rhs=WV[:, kd, :],
                        start=(kd == 0), stop=(kd == ND - 1),
                    )
                nc.vector.tensor_tensor(
                    out=VP[:, tt, 0:HPC:2, 0:DH],
                    in0=ps[:].rearrange("p (a b) -> p a b", b=DH)[:, 0:HPC:2, :],
                    in1=BV[:].rearrange("p (a b) -> p a b", b=DH)[:, 0:HPC:2, :],
                    op=mybir.AluOpType.add,
                )
                nc.vector.tensor_tensor(
                    out=VP[:, tt, 1:HPC:2, DH:2 * DH],
                    in0=ps[:].rearrange("p (a b) -> p a b", b=DH)[:, 1:HPC:2, :],
                    in1=BV[:].rearrange("p (a b) -> p a b", b=DH)[:, 1:HPC:2, :],
                    op=mybir.AluOpType.add,
                )

            # prologue: what head-pair 0 needs up front
            for ot in (0, 4):
                for tch in range(4):
                    emit_qk(ot, tch)
            for tt in range(8):
                emit_v(tt)
            # the rest is interleaved into the attention phase as PE filler
            fill = []
            fill += [("v", tt) for tt in range(8, 12)]
            fill += [("qk", 1, tch) for tch in range(4)]
            fill += [("v", tt) for tt in range(12, 16)]
            fill += [("qk", 5, tch) for tch in range(4)]
            fill += [("qk", 2, tch) for tch in range(4)]
            fill += [("qk", 6, tch) for tch in range(4)]
            fill += [("qk", 3, tch) for tch in range(4)]
            fill += [("qk", 7, tch) for tch in range(4)]
            fill = list(reversed(fill))  # pop() from the front

            def pop_fill():
                if fill:
                    u = fill.pop()
                    if u[0] == "v":
                        emit_v(u[1])
                    else:
                        emit_qk(u[1], u[2])

            # ---- attention per head; O.T accumulated with V' stationary ----
            # two heads (one even, one odd) are software-pipelined: while
            # ScalarE exps head A's scores, PE runs head B's score matmuls.
            def st_exp(h, J, pair):
                prow = (h % 2) * 64
                QTh = QKT[prow:prow + 64, h // 2, :]
                KTh = QKT[prow:prow + 64, 4 + h // 2, :]
                ps = psS.tile([128, 1024], f32, tag="s",
                              name=f"ps{h}_{J}_{pair[0]}")
                pt = ptp.tile([128, 1024], bf16, tag="p",
                              name=f"pt{h}_{J}_{pair[0]}")
                col = 0
                offs = []
                for i in pair:
                    qlo = max(J * 512, i * 128)
                    span = (J + 1) * 512 - qlo
                    # each matmul region must stay within one bank
                    assert col // 512 == (col + span - 1) // 512
                    nc.tensor.matmul(
                        ps[:, col:col + span],
                        lhsT=KTh[:, i * 128:(i + 1) * 128],
                        rhs=QTh[:, qlo:qlo + span],
                        start=True, stop=True,
                    )
                    offs.append((i, col, qlo, span))
                    col += span
                nc.scalar.activation(
                    out=pt[:, 0:col], in_=ps[:, 0:col], func=Exp, scale=0.125)
                return pt, offs

            def av(h, J, pt, offs, otr):
                for i, coff, qlo, span in offs:
                    if i >= 4 * J:  # diagonal: zero upper triangle
                        nc.vector.tensor_tensor(
                            out=pt[:, coff:coff + 128],
                            in0=pt[:, coff:coff + 128], in1=TRI[:],
                            op=mybir.AluOpType.mult,
                        )
                    # O.T[:, qloc:512] += V'_i.T @ P.T_i
                    qloc = qlo - J * 512
                    nc.tensor.matmul(
                        otr[:, qloc:512],
                        lhsT=VP[:, i, h, :],
                        rhs=pt[:, coff:coff + span],
                        start=(i == 0), stop=(i == 4 * J + 3),
                    )

            def normalize(h, J, otr):
                # O.T rows (base prow) times 1/den rows (base drow; all 64
                # denominator rows are identical by construction)
                prow = (h % 2) * 64
                drow = 64 - prow
                rdc = dnp.tile([128, 512], f32, tag="dc", name=f"rdc{h}_{J}")
                rd = dnp.tile([128, 512], f32, tag="d", name=f"rd{h}_{J}")
                # reciprocal_approx_fast only works at partition base 0
                nc.vector.tensor_copy(
                    rdc[0:64, :], otr[drow:drow + 64, :])
                nc.vector.reciprocal_approx_fast(
                    rd[0:64, :], rdc[0:64, :])
                nc.vector.tensor_tensor(
                    out=OT[prow:prow + 64, h // 2, J * 512:(J + 1) * 512],
                    in0=otr[prow:prow + 64, :],
                    in1=rd[0:64, :],
                    op=mybir.AluOpType.mult,
                )

            for hp in range(HPC // 2):
                h0, h1 = 2 * hp, 2 * hp + 1
                for J in range(T // 512):
                    otr0 = psO.tile([128, 512], f32, tag="o",
                                    name=f"otr{h0}_{J}")
                    otr1 = psO.tile([128, 512], f32, tag="o",
                                    name=f"otr{h1}_{J}")
                    ks = list(range(4 * J + 4))
                    pairs = [ks[m:m + 2] for m in range(0, len(ks), 2)]
                    for pair in pairs:
                        pt0, offs0 = st_exp(h0, J, pair)
                        pt1, offs1 = st_exp(h1, J, pair)
                        pop_fill()
                        av(h0, J, pt0, offs0, otr0)
                        av(h1, J, pt1, offs1, otr1)
                    normalize(h0, J, otr0)
                    normalize(h1, J, otr1)

            # ---- O-proj partial: out = O @ WoT + 0.5 b_o ----
            for tq in range(NT):
                for oc2 in range(D // 512):
                    ps = psQ.tile([128, 512], f32, tag="q")
                    for ct in range(OC // 128):
                        nc.tensor.matmul(
                            ps[:],
                            lhsT=OT[:, ct, tq * 128:(tq + 1) * 128],
                            rhs=WO[:, ct, oc2 * 512:(oc2 + 1) * 512],
                            start=(ct == 0), stop=(ct == OC // 128 - 1),
                        )
                    ob = ostage.tile([128, 512], f32, tag="ob")
                    nc.vector.tensor_tensor(
                        out=ob[:], in0=ps[:],
                        in1=BO[:, oc2 * 512:(oc2 + 1) * 512],
                        op=mybir.AluOpType.add,
                    )
                    nc.sync.dma_start(
                        out=out[tq * 128:(tq + 1) * 128, oc2 * 512:(oc2 + 1) * 512],
                        in_=ob[:],
                    )

            if debug:
                nc.sync.dma_start(
                    out=d_qkt[:, :], in_=QKT[:].rearrange("p a t -> p (a t)"))
                nc.sync.dma_start(
                    out=d_vp[:, :], in_=VP[:].rearrange("p a b c -> p (a b c)"))
                nc.sync.dma_start(
                    out=d_ot[:, :], in_=OT[:].rearrange("p a t -> p (a t)"))

    nc.compile()
    return nc


def _in_maps(x, W_qkv, b_qkv, W_o, b_o):
    x = np.asarray(x, np.float32)
    W_qkv = np.asarray(W_qkv, np.float32)
    b_qkv = np.asarray(b_qkv, np.float32)
    W_o = np.asarray(W_o, np.float32)
    b_o = np.asarray(b_o, np.float32)

    maps = []
    for c in range(N_CORES):
        b, hh = c // 2, c % 2
        rs = slice(hh * OC, (hh + 1) * OC)
        wq = W_qkv[0 * D:1 * D][rs]            # [512, 1024]
        wk = W_qkv[1 * D:2 * D][rs]
        wv = W_qkv[2 * D:3 * D][rs]
        wqkT = np.concatenate([wq, wk], 0).T   # [1024, 1024]
        bq = b_qkv[0 * D:1 * D][rs]
        bk = b_qkv[1 * D:2 * D][rs]
        bvv = b_qkv[2 * D:3 * D][rs]
        tri = np.triu(np.ones((128, 128), np.float32))
        maps.append({
            "xT": np.ascontiguousarray(x[b].T).astype(BF16),
            "wqkT": np.ascontiguousarray(wqkT).astype(BF16),
            "wvT": np.ascontiguousarray(wv.T).astype(BF16),
            "woT": np.ascontiguousarray(W_o[:, rs].T).astype(BF16),
            "bqk": np.concatenate([bq, bk]).reshape(2 * OC, 1),
            "bv": bvv.reshape(1, OC),
            "bo": (0.5 * b_o).reshape(1, D),
            "tri": tri.astype(BF16),
        })
    return maps


def _run(x, W_qkv, b_qkv, W_o, b_o, trace=False, tmpdir=None):
    from concourse.bass_utils import run_bass_kernel_spmd

    if "nc" not in _cache:
        _cache["nc"] = _build()
    res = run_bass_kernel_spmd(
        _cache["nc"], _in_maps(x, W_qkv, b_qkv, W_o, b_o),
        core_ids=list(range(N_CORES)), trace=trace, tmpdir=tmpdir,
    )
    out = np.empty((B, T, D), np.float32)
    for b in range(B):
        out[b] = res.results[2 * b]["out"] + res.results[2 * b + 1]["out"]
    return out, res


def kernel(x, W_qkv, b_qkv, W_o, b_o):
    out, _ = _run(x, W_qkv, b_qkv, W_o, b_o, trace=False)
    return out
